# revision 18
# baseline (speedup 1.0000x reference)
"""Block-sparse (block-diagonal local) attention head for Trainium2, 8-way
data-parallel over the batch dimension (one batch element per NeuronCore).

Contract: kernel(**inputs) takes the FULL inputs from setup_inputs() and
returns the FULL output of reference(): out [8, 4096, 128] float32.

Per-core math (batch b):
  kT = (x_b @ Wk)^T, vT = (x_b @ Wv)^T, qT = (x_b @ Wq)^T   (Dh on partitions)
  per 128-token block j:
    v_j   = transpose(vT_j)                    (PE transpose, token-major)
    sT_j  = scoresT[k,q] = sum_d kT[d,k] qT[d,q]
    PT_j  = exp(sT_j / sqrt(Dh))               (no max-subtraction; logits are
                                                O(10) here, softmax algebra is
                                                exact without it)
    o'_j  = PT_j^T @ [v_j | 1 | 1]             (ones columns give row sums)
    out_j = o'_j[:, :128] * (1 / o'_j[:, 128])

Schedule notes (v5, tuned across four NTFF profiling rounds):
  - ~20 dummy warmup matmuls on zeroed SBUF keep the PE busy from the end of
    the framework preamble so the HAM clock-gate reaches 8/8 (2.4 GHz) by the
    time real data lands (v1 ran its first ~10us of matmuls at 1.2 GHz).
  - The first super-tile runs K-MAJOR (q,k,v interleaved per k-chunk): 3x the
    matmul work per arrived x byte, matching the warm PE consumption rate to
    the HBM supply rate (sequential q,k,v outruns the DMA 2x, starves, and
    the HAM re-throttles mid-ramp).
  - All x loads ride the sync HWDGE ring in exact consumption order: one
    FIFO ring makes arrival order == issue order; splitting across rings
    lets the SDMA packet round-robin starve the critical chunk behind bulk
    traffic.  Weights arrive as per-k combined [wq|wk|wv] chunks on the
    scalar ring.  Host-side layouts are chosen for DMA run length: weights
    k-major (768 B runs), x super-tile-contiguous (8 KiB runs; v4's 256 B
    weight packets starved the x queue to half rate).  Mid-kernel output
    stores go out via gpsimd (SWDGE) so neither HWDGE ring nor the ACT
    queue is disturbed; tail stores use the then-idle HWDGE rings.
  - scores matmuls in bf16 (v1: f32r): 107 -> 57 ns each.
  - 4 v-transposes write one PSUM bank (disjoint column slices of one
    accumulation group) -> one batched DVE copy out; the ones-columns of
    the [v | 1 1] tiles are persistent (memset once).
  - normalization alternates ACT / DVE per block.
  - super-tile 7 is processed as [256, 128, 128]-token sub-tiles: the final
    serial chain (evac copy -> scores -> exp -> o' -> recip -> norm ->
    store) then operates on a single 128-token block, and the previous
    sub-tiles' chains hide under the next sub-tile's projections.
"""

import numpy as np
from contextlib import ExitStack

B, S, D, Dh, BLOCK = 8, 4096, 1024, 128, 128
KC = D // 128  # contraction chunks of 128
STS = 512      # token super-tile per x DMA
NST = S // STS
SCALE = float(1.0 / np.sqrt(np.float32(Dh)))
N_WARMUP = 20  # dummy matmuls to warm the PE clock gate

_CACHE = {}


def _build():
    import concourse.bass as bass
    import concourse.mybir as mybir
    import concourse.tile as tile
    from concourse import bacc

    f32 = mybir.dt.float32
    bf16 = mybir.dt.bfloat16
    ts = bass.ts
    Exp = mybir.ActivationFunctionType.Exp
    Copy = mybir.ActivationFunctionType.Copy

    nc = bacc.Bacc("TRN2", target_bir_lowering=False, debug=False)

    # xq[p, st, k, s] and wp[p, k, i, d]: layouts chosen so every DMA reads
    # long contiguous runs (see header).
    xq = nc.dram_tensor("xq", [128, NST, KC, STS], bf16, kind="ExternalInput").ap()
    ident_d = nc.dram_tensor("ident", [128, 128], bf16, kind="ExternalInput").ap()
    wp = nc.dram_tensor("wp", [128, KC, 3, Dh], bf16, kind="ExternalInput").ap()
    out = nc.dram_tensor("out", [S, Dh], f32, kind="ExternalOutput").ap()

    with tile.TileContext(nc) as tc, ExitStack() as ctx:
        wpool = ctx.enter_context(tc.tile_pool(name="w", bufs=1))
        cpool = ctx.enter_context(tc.tile_pool(name="const", bufs=1))
        xpool = ctx.enter_context(tc.tile_pool(name="x", bufs=4))
        spool = ctx.enter_context(tc.tile_pool(name="s", bufs=2))
        tpool = ctx.enter_context(tc.tile_pool(name="t", bufs=2))
        opool = ctx.enter_context(tc.tile_pool(name="o", bufs=3))
        rpool = ctx.enter_context(tc.tile_pool(name="r", bufs=4))
        ppool = ctx.enter_context(tc.tile_pool(name="proj_ps", bufs=2, space="PSUM"))
        spsum = ctx.enter_context(tc.tile_pool(name="s_ps", bufs=1, space="PSUM"))
        vpsum = ctx.enter_context(tc.tile_pool(name="v_ps", bufs=1, space="PSUM"))
        qpool = ctx.enter_context(tc.tile_pool(name="o_ps", bufs=4, space="PSUM"))

        # --- PE warmup: zero a small tile, then stream dummy matmuls so the
        # HAM clock-gate sees a busy PE while the DMA rings prime.
        dummy = cpool.tile([128, 128], bf16, tag="dummy")
        nc.vector.memset(dummy[:], 0.0)
        for _ in range(N_WARMUP):
            d_ps = ppool.tile([128, STS], f32, tag="proj")
            nc.tensor.matmul(d_ps[:, 0:128], dummy[:], dummy[:], start=True, stop=True)

        # --- persistent tiles
        ident = cpool.tile([128, 128], bf16, tag="ident")
        v_mts = []
        for p in range(2):
            v_mt = cpool.tile([128, 4, BLOCK + 2], bf16, tag=f"vmt{p}")
            nc.vector.memset(v_mt[:, :, BLOCK : BLOCK + 2], 1.0)
            v_mts.append(v_mt)

        # --- weight + x priming (see header).  The ramp consumes
        # (w_k, x0_k) pairs in k order at ~345 GB/s (k-major, warm), which
        # needs BOTH HWDGE rings: the pairs alternate rings in global
        # consumption order, so each pair lands together and the SDMA
        # packet round-robin across the two rings reproduces the global
        # order at combined HBM rate.  Steady state: sync x[0:4],
        # scalar x[4:8] (one ring alone tops out at ~160 GB/s, right at
        # the sustained consumption rate).
        wp_t = wpool.tile([128, KC, 3, Dh], bf16, tag="wp")

        mi = 0  # m-tile counter (for v_mt parity)

        def emit_mtile(xt, m0, moff, mtk, kmajor, last, store_eng,
                       ramp_dummies=False):
            nonlocal mi
            jt = mtk // BLOCK
            qT_sb = spool.tile([128, mtk], bf16, tag="qT")
            kT_sb = spool.tile([128, mtk], bf16, tag="kT")
            vT_sb = spool.tile([128, mtk], bf16, tag="vT")
            v_mt = v_mts[mi % 2]
            mi += 1

            def v_transposes():
                # jt PE transposes into one PSUM bank (disjoint column
                # slices of one accumulation group) + one batched copy into
                # the persistent [v | 1 1] tile.
                v_ps = vpsum.tile([128, jt, BLOCK], bf16, tag="vps")
                for j in range(jt):
                    nc.tensor.matmul(
                        v_ps[:, j],
                        vT_sb[:, ts(j, BLOCK)],
                        ident[:],
                        is_transpose=True,
                        start=(j == 0),
                        stop=(j == jt - 1),
                    )
                nc.vector.tensor_copy(v_mt[:, 0:jt, 0:BLOCK], v_ps[:])

            if kmajor:
                # q,k,v interleaved per k-chunk (ramp supply matching); the
                # third PSUM group borrows a slot from the o' pool
                q_ps = ppool.tile([128, mtk], f32, tag="proj")
                k_ps = ppool.tile([128, mtk], f32, tag="proj")
                v_ps2 = qpool.tile([128, mtk], f32, tag="ops")
                for k in range(KC):
                    for wi, pps in ((0, q_ps), (1, k_ps), (2, v_ps2)):
                        nc.tensor.matmul(
                            pps[:],
                            wp_t[:, k, wi, :],
                            xt[:, k, moff : moff + mtk],
                            start=(k == 0),
                            stop=(k == KC - 1),
                        )
                    if ramp_dummies and k < KC - 1:
                        # keep the HAM activity window dense while the ramp
                        # is delivery-bound, so the clock is at 8/8 when the
                        # DMA cushion fills
                        for _ in range(2):
                            d_ps = qpool.tile([128, 128], f32, tag="ops")
                            nc.tensor.matmul(
                                d_ps[:], dummy[:], dummy[:], start=True, stop=True
                            )
                nc.scalar.copy(kT_sb[:], k_ps[:])
                nc.vector.tensor_copy(vT_sb[:], v_ps2[:])
                v_transposes()
                nc.vector.tensor_copy(qT_sb[:], q_ps[:])
            else:
                # sequential k, v, q: v's transposes/copies and kT staging
                # overlap the q projection; after the last q matmul only
                # scores -> exp -> o' remain
                for wi, sb, copy_eng in (
                    (1, kT_sb, nc.scalar),
                    (2, vT_sb, nc.vector),
                    (0, qT_sb, nc.vector),
                ):
                    pps = ppool.tile([128, mtk], f32, tag="proj")
                    for k in range(KC):
                        nc.tensor.matmul(
                            pps[:],
                            wp_t[:, k, wi, :],
                            xt[:, k, moff : moff + mtk],
                            start=(k == 0),
                            stop=(k == KC - 1),
                        )
                    if copy_eng is nc.scalar:
                        nc.scalar.copy(sb[:], pps[:])
                    else:
                        nc.vector.tensor_copy(sb[:], pps[:])
                    if wi == 2:
                        v_transposes()

            # ---- attention
            sT_big = spsum.tile([128, jt * BLOCK], f32, tag="sT")
            for j in range(jt):
                blk = ts(j, BLOCK)
                nc.tensor.matmul(
                    sT_big[:, blk],
                    kT_sb[:, blk],
                    qT_sb[:, blk],
                    start=(j == 0),
                    stop=(j == jt - 1),
                )
            PT_big = tpool.tile([128, jt * BLOCK], bf16, tag="PT")
            nc.scalar.activation(PT_big[:], sT_big[:], Exp, scale=SCALE)

            o_mt = opool.tile([128, jt, BLOCK], f32, tag="o_mt")
            out_view = out[m0 : m0 + mtk, :].rearrange("(c p) d -> p c d", p=BLOCK)
            for j in range(jt):
                blk = ts(j, BLOCK)
                r_sb = rpool.tile([128, 1], f32, tag="r")
                if last:
                    # separate tiny sums matmul first: the reciprocal runs
                    # on DVE while the o' matmul streams, and the DVE norm
                    # starts right after o' lands
                    s_ps = qpool.tile([128, 2], f32, tag="ops")
                    nc.tensor.matmul(
                        s_ps[:], PT_big[:, blk],
                        v_mt[:, j, BLOCK : BLOCK + 2], start=True, stop=True,
                    )
                    nc.vector.reciprocal(r_sb[:], s_ps[:, 0:1])
                    o_ps = qpool.tile([128, BLOCK], f32, tag="ops")
                    nc.tensor.matmul(
                        o_ps[:], PT_big[:, blk], v_mt[:, j, 0:BLOCK],
                        start=True, stop=True,
                    )
                    nc.vector.tensor_scalar_mul(o_mt[:, j], o_ps[:], r_sb[:])
                else:
                    o_ps = qpool.tile([128, BLOCK + 2], f32, tag="ops")
                    nc.tensor.matmul(
                        o_ps[:], PT_big[:, blk], v_mt[:, j], start=True, stop=True
                    )
                    nc.vector.reciprocal(r_sb[:], o_ps[:, BLOCK : BLOCK + 1])
                    # normalize: alternate engines
                    if j % 2 == 0:
                        nc.scalar.activation(
                            o_mt[:, j], o_ps[:, 0:BLOCK], Copy, scale=r_sb[:]
                        )
                    else:
                        nc.vector.tensor_scalar_mul(
                            o_mt[:, j], o_ps[:, 0:BLOCK], r_sb[:]
                        )
            store_eng.dma_start(out_view, o_mt[:, 0:jt])

        for st in range(NST):
            s0 = st * STS
            xt = xpool.tile([128, KC, STS], bf16, tag="xt")
            if st == 0:
                # w_k0 rides sync ahead of everything (scalar's queue head
                # is the ~1.3us ACT table load); the rest of the weights go
                # on scalar in k order, x0 chunks on sync in k order.
                nc.sync.dma_start(wp_t[:, 0:1], wp[:, 0:1])
                nc.sync.dma_start(xt[:, 0:1], xq[:, st, 0:1])
                nc.scalar.dma_start(wp_t[:, 1:2], wp[:, 1:2])
                nc.sync.dma_start(xt[:, 1:2], xq[:, st, 1:2])
                nc.scalar.dma_start(wp_t[:, 2:4], wp[:, 2:4])
                nc.sync.dma_start(xt[:, 2:4], xq[:, st, 2:4])
                nc.scalar.dma_start(wp_t[:, 4:KC], wp[:, 4:KC])
                nc.sync.dma_start(xt[:, 4:6], xq[:, st, 4:6])
                nc.sync.dma_start(xt[:, 6:KC], xq[:, st, 6:KC])
                nc.scalar.dma_start(ident[:], ident_d[:])
            elif st == 1:
                nc.sync.dma_start(xt[:, 0:4], xq[:, st, 0:4])
                nc.scalar.dma_start(xt[:, 4:KC], xq[:, st, 4:KC])
            elif st == 2:
                # finer pieces: incremental arrival while the DMA cushion
                # is still building
                nc.sync.dma_start(xt[:, 0:2], xq[:, st, 0:2])
                nc.sync.dma_start(xt[:, 2:4], xq[:, st, 2:4])
                nc.scalar.dma_start(xt[:, 4:6], xq[:, st, 4:6])
                nc.scalar.dma_start(xt[:, 6:KC], xq[:, st, 6:KC])
            else:
                nc.sync.dma_start(xt[:, 0:4], xq[:, st, 0:4])
                nc.scalar.dma_start(xt[:, 4:KC], xq[:, st, 4:KC])

            if st < NST - 1:
                emit_mtile(xt, s0, 0, STS, kmajor=(st <= 1), last=False,
                           store_eng=nc.gpsimd, ramp_dummies=(st == 0))
            else:
                # final super-tile: shrinking sub-tiles so the last serial
                # chain operates on a single 128-token block.  All stores
                # except the very last go out via gpsimd: a store on the
                # scalar ring head-of-line blocks the final exp in the ACT
                # FIFO.
                emit_mtile(xt, s0, 0, 256, kmajor=False, last=False,
                           store_eng=nc.gpsimd)
                emit_mtile(xt, s0 + 256, 256, 128, kmajor=False, last=True,
                           store_eng=nc.gpsimd)
                emit_mtile(xt, s0 + 384, 384, 128, kmajor=False, last=True,
                           store_eng=nc.sync)

    nc.compile()
    return nc


def _get_nc():
    if "nc" not in _CACHE:
        _CACHE["nc"] = _build()
    return _CACHE["nc"]


def make_in_maps(x, Wq, Wk, Wv):
    import ml_dtypes

    proj_np = ml_dtypes.bfloat16
    # wp[p, k, i, d] = W_i[k*128 + p, d]  (k-major: per-k combined chunks)
    wp = np.stack(
        [np.asarray(w).reshape(KC, 128, Dh).transpose(1, 0, 2) for w in (Wq, Wk, Wv)],
        axis=2,
    )
    wp_h = np.ascontiguousarray(wp.astype(proj_np))
    ident_h = np.eye(128, dtype=proj_np)
    x = np.asarray(x)
    maps = []
    for b in range(B):
        # xq[p, st, k, s] = x[b].T[k*128 + p, st*STS + s]
        xp = np.asarray(x[b], dtype=proj_np).T.reshape(KC, 128, S).transpose(1, 0, 2)
        xqh = xp.reshape(128, KC, NST, STS).transpose(0, 2, 1, 3)
        maps.append(
            {
                "xq": np.ascontiguousarray(xqh),
                "wp": wp_h,
                "ident": ident_h,
            }
        )
    return maps


def kernel(x, Wq, Wk, Wv):
    from concourse.bass_utils import run_bass_kernel_spmd

    nc = _get_nc()
    in_maps = make_in_maps(x, Wq, Wk, Wv)
    res = run_bass_kernel_spmd(nc, in_maps, list(range(B))).results
    return np.stack([res[b]["out"] for b in range(B)], axis=0)


# revision 19
# speedup vs baseline: 1.0197x; 1.0197x over previous
"""Block-sparse (block-diagonal local) attention head for Trainium2, 8-way
data-parallel over the batch dimension (one batch element per NeuronCore).

Contract: kernel(**inputs) takes the FULL inputs from setup_inputs() and
returns the FULL output of reference(): out [8, 4096, 128] float32.

Per-core math (batch b):
  kT = (x_b @ Wk)^T, vT = (x_b @ Wv)^T, qT = (x_b @ Wq)^T   (Dh on partitions)
  per 128-token block j:
    v_j   = transpose(vT_j)                    (PE transpose, token-major)
    sT_j  = scoresT[k,q] = sum_d kT[d,k] qT[d,q]
    PT_j  = exp(sT_j / sqrt(Dh))               (no max-subtraction; logits are
                                                O(10) here, softmax algebra is
                                                exact without it)
    o'_j  = PT_j^T @ [v_j | 1 | 1]             (ones columns give row sums)
    out_j = o'_j[:, :128] * (1 / o'_j[:, 128])

Schedule notes (v5, tuned across four NTFF profiling rounds):
  - ~20 dummy warmup matmuls on zeroed SBUF keep the PE busy from the end of
    the framework preamble so the HAM clock-gate reaches 8/8 (2.4 GHz) by the
    time real data lands (v1 ran its first ~10us of matmuls at 1.2 GHz).
  - The first super-tile runs K-MAJOR (q,k,v interleaved per k-chunk): 3x the
    matmul work per arrived x byte, matching the warm PE consumption rate to
    the HBM supply rate (sequential q,k,v outruns the DMA 2x, starves, and
    the HAM re-throttles mid-ramp).
  - All x loads ride the sync HWDGE ring in exact consumption order: one
    FIFO ring makes arrival order == issue order; splitting across rings
    lets the SDMA packet round-robin starve the critical chunk behind bulk
    traffic.  Weights arrive as per-k combined [wq|wk|wv] chunks on the
    scalar ring.  Host-side layouts are chosen for DMA run length: weights
    k-major (768 B runs), x super-tile-contiguous (8 KiB runs; v4's 256 B
    weight packets starved the x queue to half rate).  Mid-kernel output
    stores go out via gpsimd (SWDGE) so neither HWDGE ring nor the ACT
    queue is disturbed; tail stores use the then-idle HWDGE rings.
  - scores matmuls in bf16 (v1: f32r): 107 -> 57 ns each.
  - 4 v-transposes write one PSUM bank (disjoint column slices of one
    accumulation group) -> one batched DVE copy out; the ones-columns of
    the [v | 1 1] tiles are persistent (memset once).
  - normalization alternates ACT / DVE per block.
  - super-tile 7 is processed as [256, 128, 128]-token sub-tiles: the final
    serial chain (evac copy -> scores -> exp -> o' -> recip -> norm ->
    store) then operates on a single 128-token block, and the previous
    sub-tiles' chains hide under the next sub-tile's projections.
"""

import numpy as np
from contextlib import ExitStack

B, S, D, Dh, BLOCK = 8, 4096, 1024, 128, 128
KC = D // 128  # contraction chunks of 128
STS = 512      # token super-tile per x DMA
NST = S // STS
SCALE = float(1.0 / np.sqrt(np.float32(Dh)))
N_WARMUP = 20  # dummy matmuls to warm the PE clock gate

_CACHE = {}


def _build():
    import concourse.bass as bass
    import concourse.mybir as mybir
    import concourse.tile as tile
    from concourse import bacc

    f32 = mybir.dt.float32
    bf16 = mybir.dt.bfloat16
    ts = bass.ts
    Exp = mybir.ActivationFunctionType.Exp
    Copy = mybir.ActivationFunctionType.Copy

    nc = bacc.Bacc("TRN2", target_bir_lowering=False, debug=False)

    # xq[p, st, k, s] and wp[p, k, i, d]: layouts chosen so every DMA reads
    # long contiguous runs (see header).
    xq = nc.dram_tensor("xq", [128, NST, KC, STS], bf16, kind="ExternalInput").ap()
    ident_d = nc.dram_tensor("ident", [128, 128], bf16, kind="ExternalInput").ap()
    wp = nc.dram_tensor("wp", [128, KC, 3, Dh], bf16, kind="ExternalInput").ap()
    out = nc.dram_tensor("out", [S, Dh], f32, kind="ExternalOutput").ap()

    with tile.TileContext(nc) as tc, ExitStack() as ctx:
        wpool = ctx.enter_context(tc.tile_pool(name="w", bufs=1))
        cpool = ctx.enter_context(tc.tile_pool(name="const", bufs=1))
        xpool = ctx.enter_context(tc.tile_pool(name="x", bufs=4))
        spool = ctx.enter_context(tc.tile_pool(name="s", bufs=2))
        tpool = ctx.enter_context(tc.tile_pool(name="t", bufs=2))
        opool = ctx.enter_context(tc.tile_pool(name="o", bufs=3))
        rpool = ctx.enter_context(tc.tile_pool(name="r", bufs=4))
        ppool = ctx.enter_context(tc.tile_pool(name="proj_ps", bufs=2, space="PSUM"))
        spsum = ctx.enter_context(tc.tile_pool(name="s_ps", bufs=1, space="PSUM"))
        vpsum = ctx.enter_context(tc.tile_pool(name="v_ps", bufs=1, space="PSUM"))
        qpool = ctx.enter_context(tc.tile_pool(name="o_ps", bufs=4, space="PSUM"))

        # --- PE warmup: zero a small tile, then stream dummy matmuls so the
        # HAM clock-gate sees a busy PE while the DMA rings prime.
        dummy = cpool.tile([128, 128], bf16, tag="dummy")
        nc.vector.memset(dummy[:], 0.0)
        for _ in range(N_WARMUP):
            d_ps = ppool.tile([128, STS], f32, tag="proj")
            nc.tensor.matmul(d_ps[:, 0:128], dummy[:], dummy[:], start=True, stop=True)

        # --- persistent tiles
        ident = cpool.tile([128, 128], bf16, tag="ident")
        v_mts = []
        for p in range(2):
            v_mt = cpool.tile([128, 4, BLOCK + 2], bf16, tag=f"vmt{p}")
            nc.vector.memset(v_mt[:, :, BLOCK : BLOCK + 2], 1.0)
            v_mts.append(v_mt)

        # --- weight + x priming (see header).  The ramp consumes
        # (w_k, x0_k) pairs in k order at ~345 GB/s (k-major, warm), which
        # needs BOTH HWDGE rings: the pairs alternate rings in global
        # consumption order, so each pair lands together and the SDMA
        # packet round-robin across the two rings reproduces the global
        # order at combined HBM rate.  Steady state: sync x[0:4],
        # scalar x[4:8] (one ring alone tops out at ~160 GB/s, right at
        # the sustained consumption rate).
        wp_t = wpool.tile([128, KC, 3, Dh], bf16, tag="wp")

        mi = 0  # m-tile counter (for v_mt parity)

        def emit_mtile(xt, m0, moff, mtk, kmajor, last, store_eng,
                       ramp_dummies=False):
            nonlocal mi
            jt = mtk // BLOCK
            qT_sb = spool.tile([128, mtk], bf16, tag="qT")
            kT_sb = spool.tile([128, mtk], bf16, tag="kT")
            vT_sb = spool.tile([128, mtk], bf16, tag="vT")
            v_mt = v_mts[mi % 2]
            mi += 1

            def v_transposes():
                # jt PE transposes into one PSUM bank (disjoint column
                # slices of one accumulation group) + one batched copy into
                # the persistent [v | 1 1] tile.
                v_ps = vpsum.tile([128, jt, BLOCK], bf16, tag="vps")
                for j in range(jt):
                    nc.tensor.matmul(
                        v_ps[:, j],
                        vT_sb[:, ts(j, BLOCK)],
                        ident[:],
                        is_transpose=True,
                        start=(j == 0),
                        stop=(j == jt - 1),
                    )
                nc.vector.tensor_copy(v_mt[:, 0:jt, 0:BLOCK], v_ps[:])

            if kmajor:
                # q,k,v interleaved per k-chunk (ramp supply matching); the
                # third PSUM group borrows a slot from the o' pool
                q_ps = ppool.tile([128, mtk], f32, tag="proj")
                k_ps = ppool.tile([128, mtk], f32, tag="proj")
                v_ps2 = qpool.tile([128, mtk], f32, tag="ops")
                for k in range(KC):
                    for wi, pps in ((0, q_ps), (1, k_ps), (2, v_ps2)):
                        nc.tensor.matmul(
                            pps[:],
                            wp_t[:, k, wi, :],
                            xt[:, k, moff : moff + mtk],
                            start=(k == 0),
                            stop=(k == KC - 1),
                        )
                nc.scalar.copy(kT_sb[:], k_ps[:])
                nc.vector.tensor_copy(vT_sb[:], v_ps2[:])
                v_transposes()
                nc.vector.tensor_copy(qT_sb[:], q_ps[:])
            else:
                # sequential k, v, q: v's transposes/copies and kT staging
                # overlap the q projection; after the last q matmul only
                # scores -> exp -> o' remain
                for wi, sb, copy_eng in (
                    (1, kT_sb, nc.scalar),
                    (2, vT_sb, nc.vector),
                    (0, qT_sb, nc.vector),
                ):
                    pps = ppool.tile([128, mtk], f32, tag="proj")
                    for k in range(KC):
                        nc.tensor.matmul(
                            pps[:],
                            wp_t[:, k, wi, :],
                            xt[:, k, moff : moff + mtk],
                            start=(k == 0),
                            stop=(k == KC - 1),
                        )
                    if copy_eng is nc.scalar:
                        nc.scalar.copy(sb[:], pps[:])
                    else:
                        nc.vector.tensor_copy(sb[:], pps[:])
                    if wi == 2:
                        v_transposes()

            # ---- attention
            sT_big = spsum.tile([128, jt * BLOCK], f32, tag="sT")
            for j in range(jt):
                blk = ts(j, BLOCK)
                nc.tensor.matmul(
                    sT_big[:, blk],
                    kT_sb[:, blk],
                    qT_sb[:, blk],
                    start=(j == 0),
                    stop=(j == jt - 1),
                )
            PT_big = tpool.tile([128, jt * BLOCK], bf16, tag="PT")
            nc.scalar.activation(PT_big[:], sT_big[:], Exp, scale=SCALE)

            o_mt = opool.tile([128, jt, BLOCK], f32, tag="o_mt")
            out_view = out[m0 : m0 + mtk, :].rearrange("(c p) d -> p c d", p=BLOCK)
            for j in range(jt):
                blk = ts(j, BLOCK)
                r_sb = rpool.tile([128, 1], f32, tag="r")
                if last:
                    # separate tiny sums matmul first: the reciprocal runs
                    # on DVE while the o' matmul streams, and the DVE norm
                    # starts right after o' lands
                    s_ps = qpool.tile([128, 2], f32, tag="ops")
                    nc.tensor.matmul(
                        s_ps[:], PT_big[:, blk],
                        v_mt[:, j, BLOCK : BLOCK + 2], start=True, stop=True,
                    )
                    nc.vector.reciprocal(r_sb[:], s_ps[:, 0:1])
                    o_ps = qpool.tile([128, BLOCK], f32, tag="ops")
                    nc.tensor.matmul(
                        o_ps[:], PT_big[:, blk], v_mt[:, j, 0:BLOCK],
                        start=True, stop=True,
                    )
                    nc.vector.tensor_scalar_mul(o_mt[:, j], o_ps[:], r_sb[:])
                else:
                    o_ps = qpool.tile([128, BLOCK + 2], f32, tag="ops")
                    nc.tensor.matmul(
                        o_ps[:], PT_big[:, blk], v_mt[:, j], start=True, stop=True
                    )
                    nc.vector.reciprocal(r_sb[:], o_ps[:, BLOCK : BLOCK + 1])
                    # normalize: alternate engines
                    if j % 2 == 0:
                        nc.scalar.activation(
                            o_mt[:, j], o_ps[:, 0:BLOCK], Copy, scale=r_sb[:]
                        )
                    else:
                        nc.vector.tensor_scalar_mul(
                            o_mt[:, j], o_ps[:, 0:BLOCK], r_sb[:]
                        )
            store_eng.dma_start(out_view, o_mt[:, 0:jt])

        for st in range(NST):
            s0 = st * STS
            xt = xpool.tile([128, KC, STS], bf16, tag="xt")
            if st == 0:
                # w_k0 rides sync ahead of everything (scalar's queue head
                # is the ~1.3us ACT table load); the rest of the weights go
                # on scalar in k order, x0 chunks on sync in k order.
                nc.sync.dma_start(wp_t[:, 0:1], wp[:, 0:1])
                nc.sync.dma_start(xt[:, 0:1], xq[:, st, 0:1])
                nc.scalar.dma_start(wp_t[:, 1:2], wp[:, 1:2])
                nc.sync.dma_start(xt[:, 1:2], xq[:, st, 1:2])
                nc.scalar.dma_start(wp_t[:, 2:4], wp[:, 2:4])
                nc.sync.dma_start(xt[:, 2:4], xq[:, st, 2:4])
                nc.scalar.dma_start(wp_t[:, 4:KC], wp[:, 4:KC])
                nc.sync.dma_start(xt[:, 4:6], xq[:, st, 4:6])
                nc.sync.dma_start(xt[:, 6:KC], xq[:, st, 6:KC])
                nc.scalar.dma_start(ident[:], ident_d[:])
            elif st == 1:
                nc.sync.dma_start(xt[:, 0:4], xq[:, st, 0:4])
                nc.scalar.dma_start(xt[:, 4:KC], xq[:, st, 4:KC])
            elif st == 2:
                # finer pieces: incremental arrival while the DMA cushion
                # is still building
                nc.sync.dma_start(xt[:, 0:2], xq[:, st, 0:2])
                nc.sync.dma_start(xt[:, 2:4], xq[:, st, 2:4])
                nc.scalar.dma_start(xt[:, 4:6], xq[:, st, 4:6])
                nc.scalar.dma_start(xt[:, 6:KC], xq[:, st, 6:KC])
            else:
                nc.sync.dma_start(xt[:, 0:4], xq[:, st, 0:4])
                nc.scalar.dma_start(xt[:, 4:KC], xq[:, st, 4:KC])

            if st < NST - 1:
                emit_mtile(xt, s0, 0, STS, kmajor=(st <= 1), last=False,
                           store_eng=nc.gpsimd, ramp_dummies=(st == 0))
            else:
                # final super-tile: shrinking sub-tiles so the last serial
                # chain operates on a single 128-token block.  All stores
                # except the very last go out via gpsimd: a store on the
                # scalar ring head-of-line blocks the final exp in the ACT
                # FIFO.
                emit_mtile(xt, s0, 0, 256, kmajor=False, last=False,
                           store_eng=nc.gpsimd)
                emit_mtile(xt, s0 + 256, 256, 128, kmajor=False, last=True,
                           store_eng=nc.gpsimd)
                emit_mtile(xt, s0 + 384, 384, 128, kmajor=False, last=True,
                           store_eng=nc.sync)

    nc.compile()
    return nc


def _get_nc():
    if "nc" not in _CACHE:
        _CACHE["nc"] = _build()
    return _CACHE["nc"]


def make_in_maps(x, Wq, Wk, Wv):
    import ml_dtypes

    proj_np = ml_dtypes.bfloat16
    # wp[p, k, i, d] = W_i[k*128 + p, d]  (k-major: per-k combined chunks)
    wp = np.stack(
        [np.asarray(w).reshape(KC, 128, Dh).transpose(1, 0, 2) for w in (Wq, Wk, Wv)],
        axis=2,
    )
    wp_h = np.ascontiguousarray(wp.astype(proj_np))
    ident_h = np.eye(128, dtype=proj_np)
    x = np.asarray(x)
    maps = []
    for b in range(B):
        # xq[p, st, k, s] = x[b].T[k*128 + p, st*STS + s]
        xp = np.asarray(x[b], dtype=proj_np).T.reshape(KC, 128, S).transpose(1, 0, 2)
        xqh = xp.reshape(128, KC, NST, STS).transpose(0, 2, 1, 3)
        maps.append(
            {
                "xq": np.ascontiguousarray(xqh),
                "wp": wp_h,
                "ident": ident_h,
            }
        )
    return maps


def kernel(x, Wq, Wk, Wv):
    from concourse.bass_utils import run_bass_kernel_spmd

    nc = _get_nc()
    in_maps = make_in_maps(x, Wq, Wk, Wv)
    res = run_bass_kernel_spmd(nc, in_maps, list(range(B))).results
    return np.stack([res[b]["out"] for b in range(B)], axis=0)
